# revision 31
# baseline (speedup 1.0000x reference)
"""Grouped linear (MoE routing) Trainium2 kernel.

y[t] = x[t] @ weight[g_t] + bias[g_t],  g_t = group_indices[t]

Data-parallel over 8 cores (8192 tokens each), weights replicated.

Routing is resolved on the host while sharding: each core's tokens are
stable-sorted by group and x is laid out contraction-major as
xt[din, slot] with group blocks padded to 128-slot tiles (pad columns
zero).  The device kernel is then a pure streaming grouped GEMM:

  1. Sequential HWDGE loads of 512-slot xt chunks (4-chunk prefetch)
     give lhsT tiles [128 din, 128 slots] with no on-chip transpose or
     gather.  x and weights are laid out per-partition-contiguous on
     the host (chunk-major) so each chunk/group load is 128 descriptors
     of 8-16 KB - a [128, 8, n] strided walk would cost 1024 small
     descriptors and ~10us of fixed DMA latency per chunk.
  2. Per 128-slot tile, 8 K-chunks x 2 N-chunks of (K=128, M=128,
     N=512) bf16 matmuls accumulate f32 in PSUM (all 8 banks in
     flight); group weights stream through SBUF double-buffered.
  3. DVE fuses bias add (pre-broadcast per group, bf16) with
     PSUM->SBUF copy; indirect_dma_start scatters each tile's 128 rows
     to out[token] using host-computed slot->token offsets (pads
     skipped via bounds_check).  Scatters round-robin across NOUT
     separate output tensors: consecutive scatters to one tensor are
     WAW-chained (desc-gen + ~2us completion latency each), so
     interleaving NOUT independent chains keeps the per-tile scatter
     pace under the PE's per-tile compute time.  (Batching >128 rows
     into one scatter via a multi-column offset AP wedges the SWDGE
     exec unit - NRT_EXEC_UNIT_UNRECOVERABLE - so more tensors, not
     bigger scatters.)  The host merges the NOUT shards row-wise while
     unsharding, using the tile->token map it computed for routing.

Output is bf16 (the reference itself accumulates in bf16); the host
upcasts to f32 while unsharding.  Back-pressure keeps the PE
continuously fed so the HAM clock stays at 2.4 GHz.
"""

import sys

import numpy as np

sys.path.insert(0, "/opt/trn_rl_repo")

from concourse import bacc, bass, mybir, tile  # noqa: E402

N_CORES = 8
BATCH = 65536
TOK = BATCH // N_CORES  # tokens per core
DIN = 1024
DOUT = 1024
NG = 8
P = 128

FP32 = mybir.dt.float32
BF16 = mybir.dt.bfloat16
I32 = mybir.dt.int32

SENTINEL = 99999  # > TOK-1: skipped by bounds_check on output scatter

Alu = mybir.AluOpType

XCH = 512  # slots per xt load chunk (1 KB per descriptor)
NOUT = 8  # independent output tensors (parallel scatter WAW chains)


def _chunks(nslots):
    """Chunk-size table: small leading chunks so the first tiles' data
    lands quickly, then XCH-slot chunks."""
    xch_n = []
    left = nslots
    for want in (P, P, 2 * P):
        n = min(want, left)
        if n:
            xch_n.append(n)
            left -= n
    while left:
        n = min(XCH, left)
        xch_n.append(n)
        left -= n
    xch_off = [0]
    for n in xch_n:
        xch_off.append(xch_off[-1] + n)
    return xch_n, xch_off


def build_kernel(cap):
    """cap[g] = static slot capacity of group g (multiple of 128, >=
    per-core count of group g on every core)."""
    cap = [int(c) for c in cap]
    assert all(c % P == 0 for c in cap) and sum(cap) % P == 0
    nslots = sum(cap)
    ntiles = nslots // P

    tile_group = []
    for g in range(NG):
        tile_group += [g] * (cap[g] // P)

    nc = bacc.Bacc(
        "TRN2",
        target_bir_lowering=False,
        debug=False,
        num_devices=N_CORES,
    )

    xc_d = nc.dram_tensor("xc", [P, (DIN // P) * nslots], BF16, kind="ExternalInput").ap()
    w_d = nc.dram_tensor("w", [NG, P, (DIN // P) * DOUT], BF16, kind="ExternalInput").ap()
    b_d = nc.dram_tensor("b", [NG, DOUT], BF16, kind="ExternalInput").ap()
    yo_d = nc.dram_tensor("yo", [P, ntiles], I32, kind="ExternalInput").ap()
    out_d = [
        nc.dram_tensor(f"out{k}", [TOK, DOUT], BF16, kind="ExternalOutput").ap()
        for k in range(NOUT)
    ]

    xch_n, xch_off = _chunks(nslots)
    n_xch = len(xch_n)

    with tile.TileContext(nc) as tc:
        with (
            tc.tile_pool(name="sbuf", bufs=1) as sb,
            tc.tile_pool(name="wpool", bufs=2) as wpool,
            tc.tile_pool(name="xpool", bufs=5) as xpool,
            tc.tile_pool(name="ypool", bufs=8) as ypool,
            tc.tile_pool(name="psum", bufs=8, space="PSUM") as psum,
        ):
            yo_sb = sb.tile([P, ntiles], I32, tag="yo")
            nc.sync.dma_start(out=yo_sb[:], in_=yo_d[:])

            NC = DIN // P

            def load_x(ch):
                n = xch_n[ch]
                s0 = xch_off[ch]
                xtile = xpool.tile([P, NC, n], BF16, tag="x")
                nc.sync.dma_start(
                    out=xtile[:],
                    in_=xc_d[:, NC * s0 : NC * (s0 + n)].rearrange(
                        "p (c s) -> p c s", c=NC
                    ),
                )
                return xtile

            def load_w(g, split=False):
                # scalar engine: separate HW queue, overlaps the sync-issued
                # x loads (the head is otherwise serialized on one queue)
                wt = wpool.tile([P, NC, DOUT], BF16, tag="w")
                w_r = w_d[g].rearrange("p (c j) -> p c j", c=NC)
                if split:
                    # halves on both HWDGE queues so w0 lands with x0
                    nc.scalar.dma_start(out=wt[:, 0:4, :], in_=w_r[:, 0:4, :])
                    nc.sync.dma_start(out=wt[:, 4:8, :], in_=w_r[:, 4:8, :])
                else:
                    nc.scalar.dma_start(out=wt[:], in_=w_r)
                return wt

            # first x chunk and first weight before everything else
            xtiles = {0: load_x(0)}
            w_sb = {0: load_w(0, split=True)}

            # ---------------- bias broadcast (bf16) ----------------
            # one contiguous load: 8 tiny single-partition loads each stall
            # the issuing engine ~5us
            ball = sb.tile([1, NG * DOUT], BF16, tag="ball")
            nc.sync.dma_start(out=ball[:], in_=b_d.rearrange("g j -> (g j)")[None, :])
            bias_rep = sb.tile([P, NG, DOUT], BF16, tag="bias_rep")
            for g in range(NG):
                nc.gpsimd.partition_broadcast(
                    bias_rep[:, g, :], ball[:, g * DOUT : (g + 1) * DOUT]
                )

            for g in range(1, NG):
                w_sb[g] = load_w(g)

            PREF = 4
            for ch in range(1, min(PREF, n_xch)):
                xtiles[ch] = load_x(ch)

            # ---------------- streaming grouped GEMM ----------------
            t = 0
            for ch in range(n_xch):
                if ch + PREF < n_xch:
                    xtiles[ch + PREF] = load_x(ch + PREF)
                xtile = xtiles.pop(ch)
                for off in range(0, xch_n[ch], P):
                    g = tile_group[t]
                    y_st = ypool.tile([P, DOUT], BF16, tag="y")
                    ps0 = psum.tile([P, 512], FP32, tag="acc")
                    ps1 = psum.tile([P, 512], FP32, tag="acc")
                    acc = [ps0, ps1]
                    # ic outer: both N-halves reuse the same stationary lhsT
                    for ic in range(DIN // P):
                        for jc in range(2):
                            nc.tensor.matmul(
                                out=acc[jc][:],
                                lhsT=xtile[:, ic, off : off + P],
                                rhs=w_sb[g][:, ic, jc * 512 : (jc + 1) * 512],
                                start=(ic == 0),
                                stop=(ic == DIN // P - 1),
                            )
                    for jc in range(2):
                        nc.vector.tensor_tensor(
                            out=y_st[:, jc * 512 : (jc + 1) * 512],
                            in0=acc[jc][:],
                            in1=bias_rep[:, g, jc * 512 : (jc + 1) * 512],
                            op=Alu.add,
                        )
                    nc.gpsimd.indirect_dma_start(
                        out=out_d[t % NOUT][:],
                        out_offset=bass.IndirectOffsetOnAxis(
                            ap=yo_sb[:, t : t + 1], axis=0
                        ),
                        in_=y_st[:],
                        in_offset=None,
                        bounds_check=TOK - 1,
                        oob_is_err=False,
                    )
                    t += 1
            assert t == ntiles

    nc.compile()
    return nc


def _plan_caps(gi: np.ndarray) -> np.ndarray:
    counts = np.zeros((N_CORES, NG), dtype=np.int64)
    for c in range(N_CORES):
        counts[c] = np.bincount(gi[c * TOK : (c + 1) * TOK], minlength=NG)
    mx = counts.max(axis=0)
    return ((mx + P - 1) // P) * P


def _route_core(x_c, gi_c, cap):
    """Sort one core's tokens by group into padded 128-slot blocks.

    Returns xt [DIN, nslots] bf16 (contraction-major, pads zero),
    yoff [P, ntiles] int32 (slot -> token, pads SENTINEL), and
    owner [TOK] (which of the NOUT output tensors holds each token)."""
    nslots = int(cap.sum())
    order = np.argsort(gi_c, kind="stable")
    counts = np.bincount(gi_c, minlength=NG)
    gbase = np.concatenate(([0], np.cumsum(cap)))[:NG]
    cstart = np.concatenate(([0], np.cumsum(counts)))[:NG]

    slot_token = np.full(nslots, -1, dtype=np.int64)
    xt = np.zeros((DIN, nslots), dtype=x_c.dtype)
    for g in range(NG):
        n = int(counts[g])
        toks = order[cstart[g] : cstart[g] + n]
        slot_token[gbase[g] : gbase[g] + n] = toks
        xt[:, gbase[g] : gbase[g] + n] = x_c[toks].T

    # chunk-major, per-partition-contiguous layout:
    # xc[p, NC*s0 + c*n + s] = xt[c*128 + p, s0 + s] for chunk (s0, n)
    xch_n, xch_off = _chunks(nslots)
    parts = []
    for n, s0 in zip(xch_n, xch_off):
        blk = xt[:, s0 : s0 + n]
        parts.append(blk.reshape(DIN // P, P, n).transpose(1, 0, 2).reshape(P, -1))
    xc = np.ascontiguousarray(np.concatenate(parts, axis=1))

    yoff = np.where(slot_token >= 0, slot_token, SENTINEL)
    yoff = np.ascontiguousarray(yoff.reshape(-1, P).T).astype(np.int32)

    real = slot_token >= 0
    owner = np.empty(TOK, dtype=np.int64)
    owner[slot_token[real]] = (np.arange(nslots) // P)[real] % NOUT
    return xc, yoff, owner


LAST_RESULTS = None  # stashed BassKernelResults for external profiling


def kernel(x, weight, bias, group_indices):
    global LAST_RESULTS
    from concourse.bass_utils import run_bass_kernel_spmd

    x = np.asarray(x)
    weight = np.asarray(weight)
    bias = np.asarray(bias)
    gi = np.ascontiguousarray(np.asarray(group_indices, dtype=np.int32))

    cap = _plan_caps(gi)
    nc = build_kernel(cap)

    # per-partition-contiguous weight layout: w2[g, p, c*DOUT + j]
    w2 = np.ascontiguousarray(
        weight.reshape(NG, DIN // P, P, DOUT).transpose(0, 2, 1, 3).reshape(NG, P, -1)
    )

    in_maps = []
    owners = []
    for c in range(N_CORES):
        xc, yoff, owner = _route_core(
            np.ascontiguousarray(x[c * TOK : (c + 1) * TOK]),
            gi[c * TOK : (c + 1) * TOK],
            cap,
        )
        in_maps.append({"xc": xc, "w": w2, "b": bias, "yo": yoff})
        owners.append(owner)
    res = run_bass_kernel_spmd(nc, in_maps, core_ids=list(range(N_CORES)))
    LAST_RESULTS = res

    out = np.empty((BATCH, DOUT), dtype=np.float32)
    for c in range(N_CORES):
        out_c = out[c * TOK : (c + 1) * TOK]
        for k in range(NOUT):
            m = owners[c] == k
            out_c[m] = res.results[c][f"out{k}"][m].astype(np.float32)
    return out


# revision 36
# speedup vs baseline: 1.1727x; 1.1727x over previous
"""Grouped linear (MoE routing) Trainium2 kernel.

y[t] = x[t] @ weight[g_t] + bias[g_t],  g_t = group_indices[t]

Data-parallel over 8 cores (8192 tokens each), weights replicated.

Routing is resolved on the host while sharding: each core's tokens are
stable-sorted by group and x is laid out contraction-major as
xt[din, slot] with group blocks padded to 128-slot tiles (pad columns
zero).  The device kernel is then a pure streaming grouped GEMM:

  1. Sequential HWDGE loads of 512-slot xt chunks (4-chunk prefetch)
     give lhsT tiles [128 din, 128 slots] with no on-chip transpose or
     gather.  The strided [128, 8, n] walk costs ~1024 1-KB
     descriptors (~10us latency per chunk, hidden by the prefetch
     depth); a host-side chunk-major layout with 8-KB descriptors was
     tried and is NET SLOWER - the long per-partition DMA bursts
     contend with PE operand reads on the SBUF ports and add ~40ns to
     every matmul.
  2. Per 128-slot tile, 8 K-chunks x 2 N-chunks of (K=128, M=128,
     N=512) bf16 matmuls accumulate f32 in PSUM (all 8 banks in
     flight); group weights stream through SBUF double-buffered.
  3. DVE fuses bias add (pre-broadcast per group, bf16) with
     PSUM->SBUF copy; indirect_dma_start scatters each tile's 128 rows
     to out[token] using host-computed slot->token offsets (pads
     skipped via bounds_check).  Scatters round-robin across NOUT
     separate output tensors: consecutive scatters to one tensor are
     WAW-chained (desc-gen + ~2us completion latency each), so
     interleaving NOUT independent chains keeps the per-tile scatter
     pace under the PE's per-tile compute time.  (Batching >128 rows
     into one scatter via a multi-column offset AP wedges the SWDGE
     exec unit - NRT_EXEC_UNIT_UNRECOVERABLE - so more tensors, not
     bigger scatters.)  The host merges the NOUT shards row-wise while
     unsharding, using the tile->token map it computed for routing.

Output is bf16 (the reference itself accumulates in bf16); the host
upcasts to f32 while unsharding.  Back-pressure keeps the PE
continuously fed so the HAM clock stays at 2.4 GHz.
"""

import sys

import numpy as np

sys.path.insert(0, "/opt/trn_rl_repo")

from concourse import bacc, bass, mybir, tile  # noqa: E402

N_CORES = 8
BATCH = 65536
TOK = BATCH // N_CORES  # tokens per core
DIN = 1024
DOUT = 1024
NG = 8
P = 128

FP32 = mybir.dt.float32
BF16 = mybir.dt.bfloat16
I32 = mybir.dt.int32

SENTINEL = 99999  # > TOK-1: skipped by bounds_check on output scatter

Alu = mybir.AluOpType

XCH = 512  # slots per xt load chunk (1 KB per descriptor)
NOUT = 8  # independent output tensors (parallel scatter WAW chains)


def _chunks(nslots):
    """Chunk-size table: small leading chunks so the first tiles' data
    lands quickly, then XCH-slot chunks."""
    xch_n = []
    left = nslots
    for want in (P, P, 2 * P):
        n = min(want, left)
        if n:
            xch_n.append(n)
            left -= n
    while left:
        n = min(XCH, left)
        xch_n.append(n)
        left -= n
    xch_off = [0]
    for n in xch_n:
        xch_off.append(xch_off[-1] + n)
    return xch_n, xch_off


def build_kernel(cap):
    """cap[g] = static slot capacity of group g (multiple of 128, >=
    per-core count of group g on every core)."""
    cap = [int(c) for c in cap]
    assert all(c % P == 0 for c in cap) and sum(cap) % P == 0
    nslots = sum(cap)
    ntiles = nslots // P

    tile_group = []
    for g in range(NG):
        tile_group += [g] * (cap[g] // P)

    nc = bacc.Bacc(
        "TRN2",
        target_bir_lowering=False,
        debug=False,
        num_devices=N_CORES,
    )

    xt_d = nc.dram_tensor("xt", [DIN, nslots], BF16, kind="ExternalInput").ap()
    w_d = nc.dram_tensor("w", [NG, DIN, DOUT], BF16, kind="ExternalInput").ap()
    b_d = nc.dram_tensor("b", [NG, DOUT], BF16, kind="ExternalInput").ap()
    yo_d = nc.dram_tensor("yo", [P, ntiles], I32, kind="ExternalInput").ap()
    out_d = [
        nc.dram_tensor(f"out{k}", [TOK, DOUT], BF16, kind="ExternalOutput").ap()
        for k in range(NOUT)
    ]

    xch_n, xch_off = _chunks(nslots)
    n_xch = len(xch_n)

    with tile.TileContext(nc) as tc:
        with (
            tc.tile_pool(name="sbuf", bufs=1) as sb,
            tc.tile_pool(name="wpool", bufs=2) as wpool,
            tc.tile_pool(name="xpool", bufs=5) as xpool,
            tc.tile_pool(name="ypool", bufs=8) as ypool,
            tc.tile_pool(name="psum", bufs=8, space="PSUM") as psum,
        ):
            yo_sb = sb.tile([P, ntiles], I32, tag="yo")
            nc.sync.dma_start(out=yo_sb[:], in_=yo_d[:])

            xt_r = xt_d.rearrange("(c p) s -> p c s", p=P)

            def load_x(ch):
                n = xch_n[ch]
                s0 = xch_off[ch]
                xtile = xpool.tile([P, DIN // P, n], BF16, tag="x")
                nc.sync.dma_start(out=xtile[:], in_=xt_r[:, :, s0 : s0 + n])
                return xtile

            def load_w(g, split=False):
                # scalar engine: separate HW queue, overlaps the sync-issued
                # x loads (the head is otherwise serialized on one queue)
                wt = wpool.tile([P, DIN // P, DOUT], BF16, tag="w")
                w_r = w_d[g].rearrange("(c p) j -> p c j", p=P)
                if split:
                    # halves on both HWDGE queues so w0 lands with x0
                    nc.scalar.dma_start(out=wt[:, 0:4, :], in_=w_r[:, 0:4, :])
                    nc.sync.dma_start(out=wt[:, 4:8, :], in_=w_r[:, 4:8, :])
                else:
                    nc.scalar.dma_start(out=wt[:], in_=w_r)
                return wt

            # first x chunk and first weight before everything else
            xtiles = {0: load_x(0)}
            w_sb = {0: load_w(0, split=True)}

            # ---------------- bias broadcast (bf16) ----------------
            # one contiguous load: 8 tiny single-partition loads each stall
            # the issuing engine ~5us
            ball = sb.tile([1, NG * DOUT], BF16, tag="ball")
            nc.sync.dma_start(out=ball[:], in_=b_d.rearrange("g j -> (g j)")[None, :])
            bias_rep = sb.tile([P, NG, DOUT], BF16, tag="bias_rep")
            for g in range(NG):
                nc.gpsimd.partition_broadcast(
                    bias_rep[:, g, :], ball[:, g * DOUT : (g + 1) * DOUT]
                )

            for g in range(1, NG):
                w_sb[g] = load_w(g)

            PREF = 4
            for ch in range(1, min(PREF, n_xch)):
                xtiles[ch] = load_x(ch)

            # ---------------- streaming grouped GEMM ----------------
            t = 0
            for ch in range(n_xch):
                if ch + PREF < n_xch:
                    xtiles[ch + PREF] = load_x(ch + PREF)
                xtile = xtiles.pop(ch)
                for off in range(0, xch_n[ch], P):
                    g = tile_group[t]
                    y_st = ypool.tile([P, DOUT], BF16, tag="y")
                    ps0 = psum.tile([P, 512], FP32, tag="acc")
                    ps1 = psum.tile([P, 512], FP32, tag="acc")
                    acc = [ps0, ps1]
                    # ic outer: both N-halves reuse the same stationary lhsT
                    for ic in range(DIN // P):
                        for jc in range(2):
                            nc.tensor.matmul(
                                out=acc[jc][:],
                                lhsT=xtile[:, ic, off : off + P],
                                rhs=w_sb[g][:, ic, jc * 512 : (jc + 1) * 512],
                                start=(ic == 0),
                                stop=(ic == DIN // P - 1),
                            )
                    for jc in range(2):
                        nc.vector.tensor_tensor(
                            out=y_st[:, jc * 512 : (jc + 1) * 512],
                            in0=acc[jc][:],
                            in1=bias_rep[:, g, jc * 512 : (jc + 1) * 512],
                            op=Alu.add,
                        )
                    nc.gpsimd.indirect_dma_start(
                        out=out_d[t % NOUT][:],
                        out_offset=bass.IndirectOffsetOnAxis(
                            ap=yo_sb[:, t : t + 1], axis=0
                        ),
                        in_=y_st[:],
                        in_offset=None,
                        bounds_check=TOK - 1,
                        oob_is_err=False,
                    )
                    t += 1
            assert t == ntiles

    nc.compile()
    return nc


def _plan_caps(gi: np.ndarray) -> np.ndarray:
    counts = np.zeros((N_CORES, NG), dtype=np.int64)
    for c in range(N_CORES):
        counts[c] = np.bincount(gi[c * TOK : (c + 1) * TOK], minlength=NG)
    mx = counts.max(axis=0)
    return ((mx + P - 1) // P) * P


def _route_core(x_c, gi_c, cap):
    """Sort one core's tokens by group into padded 128-slot blocks.

    Returns xt [DIN, nslots] bf16 (contraction-major, pads zero),
    yoff [P, ntiles] int32 (slot -> token, pads SENTINEL), and
    owner [TOK] (which of the NOUT output tensors holds each token)."""
    nslots = int(cap.sum())
    order = np.argsort(gi_c, kind="stable")
    counts = np.bincount(gi_c, minlength=NG)
    gbase = np.concatenate(([0], np.cumsum(cap)))[:NG]
    cstart = np.concatenate(([0], np.cumsum(counts)))[:NG]

    slot_token = np.full(nslots, -1, dtype=np.int64)
    xt = np.zeros((DIN, nslots), dtype=x_c.dtype)
    for g in range(NG):
        n = int(counts[g])
        toks = order[cstart[g] : cstart[g] + n]
        slot_token[gbase[g] : gbase[g] + n] = toks
        xt[:, gbase[g] : gbase[g] + n] = x_c[toks].T

    yoff = np.where(slot_token >= 0, slot_token, SENTINEL)
    yoff = np.ascontiguousarray(yoff.reshape(-1, P).T).astype(np.int32)

    real = slot_token >= 0
    owner = np.empty(TOK, dtype=np.int64)
    owner[slot_token[real]] = (np.arange(nslots) // P)[real] % NOUT
    return np.ascontiguousarray(xt), yoff, owner


LAST_RESULTS = None  # stashed BassKernelResults for external profiling


def kernel(x, weight, bias, group_indices):
    global LAST_RESULTS
    from concourse.bass_utils import run_bass_kernel_spmd

    x = np.asarray(x)
    weight = np.asarray(weight)
    bias = np.asarray(bias)
    gi = np.ascontiguousarray(np.asarray(group_indices, dtype=np.int32))

    cap = _plan_caps(gi)
    nc = build_kernel(cap)

    in_maps = []
    owners = []
    for c in range(N_CORES):
        xt, yoff, owner = _route_core(
            np.ascontiguousarray(x[c * TOK : (c + 1) * TOK]),
            gi[c * TOK : (c + 1) * TOK],
            cap,
        )
        in_maps.append({"xt": xt, "w": weight, "b": bias, "yo": yoff})
        owners.append(owner)
    res = run_bass_kernel_spmd(nc, in_maps, core_ids=list(range(N_CORES)))
    LAST_RESULTS = res

    out = np.empty((BATCH, DOUT), dtype=np.float32)
    for c in range(N_CORES):
        out_c = out[c * TOK : (c + 1) * TOK]
        for k in range(NOUT):
            m = owners[c] == k
            out_c[m] = res.results[c][f"out{k}"][m].astype(np.float32)
    return out


# revision 41
# speedup vs baseline: 1.1871x; 1.0123x over previous
"""Grouped linear (MoE routing) Trainium2 kernel.

y[t] = x[t] @ weight[g_t] + bias[g_t],  g_t = group_indices[t]

Data-parallel over 8 cores (8192 tokens each), weights replicated.

Routing is resolved on the host while sharding: each core's tokens are
stable-sorted by group and x is laid out contraction-major as
xt[din, slot] with group blocks padded to 128-slot tiles (pad columns
zero).  The device kernel is then a pure streaming grouped GEMM:

  1. Sequential HWDGE loads of 512-slot xt chunks (4-chunk prefetch)
     give lhsT tiles [128 din, 128 slots] with no on-chip transpose or
     gather.  The strided [128, 8, n] walk costs ~1024 1-KB
     descriptors (~10us latency per chunk, hidden by the prefetch
     depth); a host-side chunk-major layout with 8-KB descriptors was
     tried and is NET SLOWER - the long per-partition DMA bursts
     contend with PE operand reads on the SBUF ports and add ~40ns to
     every matmul.
  2. Per 128-slot tile, 8 K-chunks x 2 N-chunks of (K=128, M=128,
     N=512) bf16 matmuls accumulate f32 in PSUM (all 8 banks in
     flight); group weights stream through SBUF double-buffered.
  3. DVE fuses bias add (pre-broadcast per group, bf16) with
     PSUM->SBUF copy; indirect_dma_start scatters each tile's 128 rows
     to out[token] using host-computed slot->token offsets (pads
     skipped via bounds_check).  Scatters round-robin across NOUT
     separate output tensors: consecutive scatters to one tensor are
     WAW-chained (desc-gen + ~2us completion latency each), so
     interleaving NOUT independent chains keeps the per-tile scatter
     pace under the PE's per-tile compute time.  (Batching >128 rows
     into one scatter via a multi-column offset AP wedges the SWDGE
     exec unit - NRT_EXEC_UNIT_UNRECOVERABLE - so more tensors, not
     bigger scatters.)  The host merges the NOUT shards row-wise while
     unsharding, using the tile->token map it computed for routing.

Output is bf16 (the reference itself accumulates in bf16); the host
upcasts to f32 while unsharding.  Back-pressure keeps the PE
continuously fed so the HAM clock stays at 2.4 GHz.
"""

import sys

import numpy as np

sys.path.insert(0, "/opt/trn_rl_repo")

from concourse import bacc, bass, mybir, tile  # noqa: E402

N_CORES = 8
BATCH = 65536
TOK = BATCH // N_CORES  # tokens per core
DIN = 1024
DOUT = 1024
NG = 8
P = 128

FP32 = mybir.dt.float32
BF16 = mybir.dt.bfloat16
I32 = mybir.dt.int32

SENTINEL = 99999  # > TOK-1: skipped by bounds_check on output scatter

Alu = mybir.AluOpType

XCH = 512  # slots per xt load chunk (1 KB per descriptor)
NOUT = 8  # independent output tensors (parallel scatter WAW chains)
HEAD = 512  # leading slots shipped twice (also chunk-major for a fast head)
N_HEAD_CH = 3  # chunks covering HEAD slots (128 + 128 + 256)


def _chunks(nslots):
    """Chunk-size table: small leading chunks so the first tiles' data
    lands quickly, then XCH-slot chunks."""
    xch_n = []
    left = nslots
    for want in (P, P, 2 * P):
        n = min(want, left)
        if n:
            xch_n.append(n)
            left -= n
    while left:
        n = min(XCH, left)
        xch_n.append(n)
        left -= n
    xch_off = [0]
    for n in xch_n:
        xch_off.append(xch_off[-1] + n)
    return xch_n, xch_off


def build_kernel(cap):
    """cap[g] = static slot capacity of group g (multiple of 128, >=
    per-core count of group g on every core)."""
    cap = [int(c) for c in cap]
    assert all(c % P == 0 for c in cap) and sum(cap) % P == 0
    nslots = sum(cap)
    ntiles = nslots // P

    tile_group = []
    for g in range(NG):
        tile_group += [g] * (cap[g] // P)

    nc = bacc.Bacc(
        "TRN2",
        target_bir_lowering=False,
        debug=False,
        num_devices=N_CORES,
    )

    xt_d = nc.dram_tensor("xt", [DIN, nslots], BF16, kind="ExternalInput").ap()
    # first HEAD slots duplicated in chunk-major per-partition-contiguous
    # form: 128 descriptors per head chunk instead of 1024, so the first
    # tiles' data lands ~8us sooner (no PE running yet, so the big-burst
    # SBUF port contention that made this layout a loss mid-stream is moot)
    xh_d = nc.dram_tensor(
        "xh", [P, (DIN // P) * HEAD], BF16, kind="ExternalInput"
    ).ap()
    w_d = nc.dram_tensor("w", [NG, DIN, DOUT], BF16, kind="ExternalInput").ap()
    b_d = nc.dram_tensor("b", [NG, DOUT], BF16, kind="ExternalInput").ap()
    yo_d = nc.dram_tensor("yo", [P, ntiles], I32, kind="ExternalInput").ap()
    out_d = [
        nc.dram_tensor(f"out{k}", [TOK, DOUT], BF16, kind="ExternalOutput").ap()
        for k in range(NOUT)
    ]

    xch_n, xch_off = _chunks(nslots)
    n_xch = len(xch_n)

    with tile.TileContext(nc) as tc:
        with (
            tc.tile_pool(name="sbuf", bufs=1) as sb,
            tc.tile_pool(name="wpool", bufs=2) as wpool,
            tc.tile_pool(name="xpool", bufs=5) as xpool,
            tc.tile_pool(name="ypool", bufs=8) as ypool,
            tc.tile_pool(name="psum", bufs=8, space="PSUM") as psum,
        ):
            yo_sb = sb.tile([P, ntiles], I32, tag="yo")
            nc.sync.dma_start(out=yo_sb[:], in_=yo_d[:])

            xt_r = xt_d.rearrange("(c p) s -> p c s", p=P)
            NC = DIN // P

            def load_x(ch):
                n = xch_n[ch]
                s0 = xch_off[ch]
                xtile = xpool.tile([P, NC, n], BF16, tag="x")
                if ch < N_HEAD_CH:
                    nc.sync.dma_start(
                        out=xtile[:],
                        in_=xh_d[:, NC * s0 : NC * (s0 + n)].rearrange(
                            "p (c s) -> p c s", c=NC
                        ),
                    )
                else:
                    nc.sync.dma_start(out=xtile[:], in_=xt_r[:, :, s0 : s0 + n])
                return xtile

            def load_w(g, split=False):
                # scalar engine: separate HW queue, overlaps the sync-issued
                # x loads (the head is otherwise serialized on one queue)
                wt = wpool.tile([P, DIN // P, DOUT], BF16, tag="w")
                w_r = w_d[g].rearrange("(c p) j -> p c j", p=P)
                if split:
                    # halves on both HWDGE queues so w0 lands with x0
                    nc.scalar.dma_start(out=wt[:, 0:4, :], in_=w_r[:, 0:4, :])
                    nc.sync.dma_start(out=wt[:, 4:8, :], in_=w_r[:, 4:8, :])
                else:
                    nc.scalar.dma_start(out=wt[:], in_=w_r)
                return wt

            # first x chunk and first weight before everything else
            xtiles = {0: load_x(0)}
            w_sb = {0: load_w(0, split=True)}

            # ---------------- bias broadcast (bf16) ----------------
            # one contiguous load: 8 tiny single-partition loads each stall
            # the issuing engine ~5us
            ball = sb.tile([1, NG * DOUT], BF16, tag="ball")
            nc.sync.dma_start(out=ball[:], in_=b_d.rearrange("g j -> (g j)")[None, :])
            bias_rep = sb.tile([P, NG, DOUT], BF16, tag="bias_rep")
            for g in range(NG):
                nc.gpsimd.partition_broadcast(
                    bias_rep[:, g, :], ball[:, g * DOUT : (g + 1) * DOUT]
                )

            for g in range(1, NG):
                w_sb[g] = load_w(g)

            PREF = 4
            for ch in range(1, min(PREF, n_xch)):
                xtiles[ch] = load_x(ch)

            # ---------------- streaming grouped GEMM ----------------
            t = 0
            for ch in range(n_xch):
                if ch + PREF < n_xch:
                    xtiles[ch + PREF] = load_x(ch + PREF)
                xtile = xtiles.pop(ch)
                for off in range(0, xch_n[ch], P):
                    g = tile_group[t]
                    y_st = ypool.tile([P, DOUT], BF16, tag="y")
                    ps0 = psum.tile([P, 512], FP32, tag="acc")
                    ps1 = psum.tile([P, 512], FP32, tag="acc")
                    acc = [ps0, ps1]
                    # ic outer: both N-halves reuse the same stationary lhsT
                    for ic in range(DIN // P):
                        for jc in range(2):
                            nc.tensor.matmul(
                                out=acc[jc][:],
                                lhsT=xtile[:, ic, off : off + P],
                                rhs=w_sb[g][:, ic, jc * 512 : (jc + 1) * 512],
                                start=(ic == 0),
                                stop=(ic == DIN // P - 1),
                            )
                    for jc in range(2):
                        nc.vector.tensor_tensor(
                            out=y_st[:, jc * 512 : (jc + 1) * 512],
                            in0=acc[jc][:],
                            in1=bias_rep[:, g, jc * 512 : (jc + 1) * 512],
                            op=Alu.add,
                        )
                    nc.gpsimd.indirect_dma_start(
                        out=out_d[t % NOUT][:],
                        out_offset=bass.IndirectOffsetOnAxis(
                            ap=yo_sb[:, t : t + 1], axis=0
                        ),
                        in_=y_st[:],
                        in_offset=None,
                        bounds_check=TOK - 1,
                        oob_is_err=False,
                    )
                    t += 1
            assert t == ntiles

    nc.compile()
    return nc


def _plan_caps(gi: np.ndarray) -> np.ndarray:
    counts = np.zeros((N_CORES, NG), dtype=np.int64)
    for c in range(N_CORES):
        counts[c] = np.bincount(gi[c * TOK : (c + 1) * TOK], minlength=NG)
    mx = counts.max(axis=0)
    return ((mx + P - 1) // P) * P


def _route_core(x_c, gi_c, cap):
    """Sort one core's tokens by group into padded 128-slot blocks.

    Returns xt [DIN, nslots] bf16 (contraction-major, pads zero),
    yoff [P, ntiles] int32 (slot -> token, pads SENTINEL), and
    owner [TOK] (which of the NOUT output tensors holds each token)."""
    nslots = int(cap.sum())
    order = np.argsort(gi_c, kind="stable")
    counts = np.bincount(gi_c, minlength=NG)
    gbase = np.concatenate(([0], np.cumsum(cap)))[:NG]
    cstart = np.concatenate(([0], np.cumsum(counts)))[:NG]

    slot_token = np.full(nslots, -1, dtype=np.int64)
    xt = np.zeros((DIN, nslots), dtype=x_c.dtype)
    for g in range(NG):
        n = int(counts[g])
        toks = order[cstart[g] : cstart[g] + n]
        slot_token[gbase[g] : gbase[g] + n] = toks
        xt[:, gbase[g] : gbase[g] + n] = x_c[toks].T

    # chunk-major duplicate of the first HEAD slots:
    # xh[p, NC*s0 + c*n + s] = xt[c*128 + p, s0 + s] per head chunk (s0, n)
    xch_n, xch_off = _chunks(nslots)
    parts = []
    for n, s0 in zip(xch_n[:N_HEAD_CH], xch_off[:N_HEAD_CH]):
        blk = xt[:, s0 : s0 + n]
        parts.append(blk.reshape(DIN // P, P, n).transpose(1, 0, 2).reshape(P, -1))
    xh = np.ascontiguousarray(np.concatenate(parts, axis=1))

    yoff = np.where(slot_token >= 0, slot_token, SENTINEL)
    yoff = np.ascontiguousarray(yoff.reshape(-1, P).T).astype(np.int32)

    real = slot_token >= 0
    owner = np.empty(TOK, dtype=np.int64)
    owner[slot_token[real]] = (np.arange(nslots) // P)[real] % NOUT
    return np.ascontiguousarray(xt), xh, yoff, owner


LAST_RESULTS = None  # stashed BassKernelResults for external profiling


def kernel(x, weight, bias, group_indices):
    global LAST_RESULTS
    from concourse.bass_utils import run_bass_kernel_spmd

    x = np.asarray(x)
    weight = np.asarray(weight)
    bias = np.asarray(bias)
    gi = np.ascontiguousarray(np.asarray(group_indices, dtype=np.int32))

    cap = _plan_caps(gi)
    nc = build_kernel(cap)

    in_maps = []
    owners = []
    for c in range(N_CORES):
        xt, xh, yoff, owner = _route_core(
            np.ascontiguousarray(x[c * TOK : (c + 1) * TOK]),
            gi[c * TOK : (c + 1) * TOK],
            cap,
        )
        in_maps.append({"xt": xt, "xh": xh, "w": weight, "b": bias, "yo": yoff})
        owners.append(owner)
    res = run_bass_kernel_spmd(nc, in_maps, core_ids=list(range(N_CORES)))
    LAST_RESULTS = res

    out = np.empty((BATCH, DOUT), dtype=np.float32)
    for c in range(N_CORES):
        out_c = out[c * TOK : (c + 1) * TOK]
        for k in range(NOUT):
            m = owners[c] == k
            out_c[m] = res.results[c][f"out{k}"][m].astype(np.float32)
    return out


# revision 45
# speedup vs baseline: 1.1915x; 1.0037x over previous
"""Grouped linear (MoE routing) Trainium2 kernel.

y[t] = x[t] @ weight[g_t] + bias[g_t],  g_t = group_indices[t]

Data-parallel over 8 cores (8192 tokens each), weights replicated.

Routing is resolved on the host while sharding: each core's tokens are
stable-sorted by group and x is laid out contraction-major as
xt[din, slot] with group blocks padded to 128-slot tiles (pad columns
zero).  The device kernel is then a pure streaming grouped GEMM:

  1. Sequential HWDGE loads of 512-slot xt chunks (4-chunk prefetch)
     give lhsT tiles [128 din, 128 slots] with no on-chip transpose or
     gather.  The strided [128, 8, n] walk costs ~1024 1-KB
     descriptors (~10us latency per chunk, hidden by the prefetch
     depth); a host-side chunk-major layout with 8-KB descriptors was
     tried and is NET SLOWER - the long per-partition DMA bursts
     contend with PE operand reads on the SBUF ports and add ~40ns to
     every matmul.
  2. Per 128-slot tile, 8 K-chunks x 2 N-chunks of (K=128, M=128,
     N=512) bf16 matmuls accumulate f32 in PSUM (all 8 banks in
     flight); group weights stream through SBUF double-buffered.
  3. DVE fuses bias add (pre-broadcast per group, bf16) with
     PSUM->SBUF copy; indirect_dma_start scatters each tile's 128 rows
     to out[token] using host-computed slot->token offsets (pads
     skipped via bounds_check).  Scatters round-robin across NOUT
     separate output tensors: consecutive scatters to one tensor are
     WAW-chained (desc-gen + ~2us completion latency each), so
     interleaving NOUT independent chains keeps the per-tile scatter
     pace under the PE's per-tile compute time.  (Batching >128 rows
     into one scatter via a multi-column offset AP wedges the SWDGE
     exec unit - NRT_EXEC_UNIT_UNRECOVERABLE - so more tensors, not
     bigger scatters.)  The host merges the NOUT shards row-wise while
     unsharding, using the tile->token map it computed for routing.

Output is bf16 (the reference itself accumulates in bf16); the host
upcasts to f32 while unsharding.  Back-pressure keeps the PE
continuously fed so the HAM clock stays at 2.4 GHz.
"""

import sys

import numpy as np

sys.path.insert(0, "/opt/trn_rl_repo")

from concourse import bacc, bass, mybir, tile  # noqa: E402

N_CORES = 8
BATCH = 65536
TOK = BATCH // N_CORES  # tokens per core
DIN = 1024
DOUT = 1024
NG = 8
P = 128

FP32 = mybir.dt.float32
BF16 = mybir.dt.bfloat16
I32 = mybir.dt.int32

SENTINEL = 99999  # > TOK-1: skipped by bounds_check on output scatter

Alu = mybir.AluOpType

XCH = 512  # slots per xt load chunk (1 KB per descriptor)
NOUT = 8  # independent output tensors (parallel scatter WAW chains)
HEAD = 512  # leading slots shipped twice (also chunk-major for a fast head)
N_HEAD_CH = 3  # chunks covering HEAD slots (128 + 128 + 256)


def _chunks(nslots):
    """Chunk-size table: small leading chunks so the first tiles' data
    lands quickly, then XCH-slot chunks."""
    xch_n = []
    left = nslots
    for want in (P, P, 2 * P):
        n = min(want, left)
        if n:
            xch_n.append(n)
            left -= n
    while left:
        n = min(XCH, left)
        xch_n.append(n)
        left -= n
    xch_off = [0]
    for n in xch_n:
        xch_off.append(xch_off[-1] + n)
    return xch_n, xch_off


def build_kernel(cap):
    """cap[g] = static slot capacity of group g (multiple of 128, >=
    per-core count of group g on every core)."""
    cap = [int(c) for c in cap]
    assert all(c % P == 0 for c in cap) and sum(cap) % P == 0
    nslots = sum(cap)
    ntiles = nslots // P

    tile_group = []
    for g in range(NG):
        tile_group += [g] * (cap[g] // P)

    nc = bacc.Bacc(
        "TRN2",
        target_bir_lowering=False,
        debug=False,
        num_devices=N_CORES,
    )

    xt_d = nc.dram_tensor("xt", [DIN, nslots], BF16, kind="ExternalInput").ap()
    # first HEAD slots duplicated in chunk-major per-partition-contiguous
    # form: 128 descriptors per head chunk instead of 1024, so the first
    # tiles' data lands ~8us sooner (no PE running yet, so the big-burst
    # SBUF port contention that made this layout a loss mid-stream is moot)
    xh_d = nc.dram_tensor(
        "xh", [P, (DIN // P) * HEAD], BF16, kind="ExternalInput"
    ).ap()
    # group-0 weight, also chunk-major: one 128-descriptor load instead of
    # two 512-descriptor halves; w0 is what the first matmul waits on
    wh_d = nc.dram_tensor(
        "wh", [P, (DIN // P) * DOUT], BF16, kind="ExternalInput"
    ).ap()
    w_d = nc.dram_tensor("w", [NG, DIN, DOUT], BF16, kind="ExternalInput").ap()
    b_d = nc.dram_tensor("b", [NG, DOUT], BF16, kind="ExternalInput").ap()
    yo_d = nc.dram_tensor("yo", [P, ntiles], I32, kind="ExternalInput").ap()
    out_d = [
        nc.dram_tensor(f"out{k}", [TOK, DOUT], BF16, kind="ExternalOutput").ap()
        for k in range(NOUT)
    ]

    xch_n, xch_off = _chunks(nslots)
    n_xch = len(xch_n)

    with tile.TileContext(nc) as tc:
        with (
            tc.tile_pool(name="sbuf", bufs=1) as sb,
            tc.tile_pool(name="wpool", bufs=2) as wpool,
            tc.tile_pool(name="xpool", bufs=5) as xpool,
            tc.tile_pool(name="ypool", bufs=8) as ypool,
            tc.tile_pool(name="psum", bufs=8, space="PSUM") as psum,
        ):
            yo_sb = sb.tile([P, ntiles], I32, tag="yo")
            nc.sync.dma_start(out=yo_sb[:], in_=yo_d[:])

            xt_r = xt_d.rearrange("(c p) s -> p c s", p=P)
            NC = DIN // P

            def load_x(ch):
                n = xch_n[ch]
                s0 = xch_off[ch]
                xtile = xpool.tile([P, NC, n], BF16, tag="x")
                if ch < N_HEAD_CH:
                    nc.sync.dma_start(
                        out=xtile[:],
                        in_=xh_d[:, NC * s0 : NC * (s0 + n)].rearrange(
                            "p (c s) -> p c s", c=NC
                        ),
                    )
                else:
                    nc.sync.dma_start(out=xtile[:], in_=xt_r[:, :, s0 : s0 + n])
                return xtile

            def load_w(g, head=False):
                # scalar engine: separate HW queue, overlaps the sync-issued
                # x loads (the head is otherwise serialized on one queue)
                wt = wpool.tile([P, DIN // P, DOUT], BF16, tag="w")
                if head:
                    nc.scalar.dma_start(
                        out=wt[:],
                        in_=wh_d.rearrange("p (c j) -> p c j", c=DIN // P),
                    )
                else:
                    nc.scalar.dma_start(
                        out=wt[:], in_=w_d[g].rearrange("(c p) j -> p c j", p=P)
                    )
                return wt

            # first x chunk and first weight before everything else
            xtiles = {0: load_x(0)}
            w_sb = {0: load_w(0, head=True)}

            # ---------------- bias broadcast (bf16) ----------------
            # one contiguous load: 8 tiny single-partition loads each stall
            # the issuing engine ~5us
            ball = sb.tile([1, NG * DOUT], BF16, tag="ball")
            nc.sync.dma_start(out=ball[:], in_=b_d.rearrange("g j -> (g j)")[None, :])
            bias_rep = sb.tile([P, NG, DOUT], BF16, tag="bias_rep")
            for g in range(NG):
                nc.gpsimd.partition_broadcast(
                    bias_rep[:, g, :], ball[:, g * DOUT : (g + 1) * DOUT]
                )

            for g in range(1, NG):
                w_sb[g] = load_w(g)

            PREF = 4
            for ch in range(1, min(PREF, n_xch)):
                xtiles[ch] = load_x(ch)

            # ---------------- streaming grouped GEMM ----------------
            t = 0
            for ch in range(n_xch):
                if ch + PREF < n_xch:
                    xtiles[ch + PREF] = load_x(ch + PREF)
                xtile = xtiles.pop(ch)
                for off in range(0, xch_n[ch], P):
                    g = tile_group[t]
                    y_st = ypool.tile([P, DOUT], BF16, tag="y")
                    ps0 = psum.tile([P, 512], FP32, tag="acc")
                    ps1 = psum.tile([P, 512], FP32, tag="acc")
                    acc = [ps0, ps1]
                    # ic outer: both N-halves reuse the same stationary lhsT
                    for ic in range(DIN // P):
                        for jc in range(2):
                            nc.tensor.matmul(
                                out=acc[jc][:],
                                lhsT=xtile[:, ic, off : off + P],
                                rhs=w_sb[g][:, ic, jc * 512 : (jc + 1) * 512],
                                start=(ic == 0),
                                stop=(ic == DIN // P - 1),
                            )
                    for jc in range(2):
                        nc.vector.tensor_tensor(
                            out=y_st[:, jc * 512 : (jc + 1) * 512],
                            in0=acc[jc][:],
                            in1=bias_rep[:, g, jc * 512 : (jc + 1) * 512],
                            op=Alu.add,
                        )
                    nc.gpsimd.indirect_dma_start(
                        out=out_d[t % NOUT][:],
                        out_offset=bass.IndirectOffsetOnAxis(
                            ap=yo_sb[:, t : t + 1], axis=0
                        ),
                        in_=y_st[:],
                        in_offset=None,
                        bounds_check=TOK - 1,
                        oob_is_err=False,
                    )
                    t += 1
            assert t == ntiles

    nc.compile()
    return nc


def _plan_caps(gi: np.ndarray) -> np.ndarray:
    counts = np.zeros((N_CORES, NG), dtype=np.int64)
    for c in range(N_CORES):
        counts[c] = np.bincount(gi[c * TOK : (c + 1) * TOK], minlength=NG)
    mx = counts.max(axis=0)
    return ((mx + P - 1) // P) * P


def _route_core(x_c, gi_c, cap):
    """Sort one core's tokens by group into padded 128-slot blocks.

    Returns xt [DIN, nslots] bf16 (contraction-major, pads zero),
    yoff [P, ntiles] int32 (slot -> token, pads SENTINEL), and
    owner [TOK] (which of the NOUT output tensors holds each token)."""
    nslots = int(cap.sum())
    order = np.argsort(gi_c, kind="stable")
    counts = np.bincount(gi_c, minlength=NG)
    gbase = np.concatenate(([0], np.cumsum(cap)))[:NG]
    cstart = np.concatenate(([0], np.cumsum(counts)))[:NG]

    slot_token = np.full(nslots, -1, dtype=np.int64)
    xt = np.zeros((DIN, nslots), dtype=x_c.dtype)
    for g in range(NG):
        n = int(counts[g])
        toks = order[cstart[g] : cstart[g] + n]
        slot_token[gbase[g] : gbase[g] + n] = toks
        xt[:, gbase[g] : gbase[g] + n] = x_c[toks].T

    # chunk-major duplicate of the first HEAD slots:
    # xh[p, NC*s0 + c*n + s] = xt[c*128 + p, s0 + s] per head chunk (s0, n)
    xch_n, xch_off = _chunks(nslots)
    parts = []
    for n, s0 in zip(xch_n[:N_HEAD_CH], xch_off[:N_HEAD_CH]):
        blk = xt[:, s0 : s0 + n]
        parts.append(blk.reshape(DIN // P, P, n).transpose(1, 0, 2).reshape(P, -1))
    xh = np.ascontiguousarray(np.concatenate(parts, axis=1))

    yoff = np.where(slot_token >= 0, slot_token, SENTINEL)
    yoff = np.ascontiguousarray(yoff.reshape(-1, P).T).astype(np.int32)

    real = slot_token >= 0
    owner = np.empty(TOK, dtype=np.int64)
    owner[slot_token[real]] = (np.arange(nslots) // P)[real] % NOUT
    return np.ascontiguousarray(xt), xh, yoff, owner


LAST_RESULTS = None  # stashed BassKernelResults for external profiling


def kernel(x, weight, bias, group_indices):
    global LAST_RESULTS
    from concourse.bass_utils import run_bass_kernel_spmd

    x = np.asarray(x)
    weight = np.asarray(weight)
    bias = np.asarray(bias)
    gi = np.ascontiguousarray(np.asarray(group_indices, dtype=np.int32))

    cap = _plan_caps(gi)
    nc = build_kernel(cap)

    # chunk-major group-0 weight: wh[p, c*DOUT + j] = weight[0, c*128+p, j]
    wh = np.ascontiguousarray(
        weight[0].reshape(DIN // P, P, DOUT).transpose(1, 0, 2).reshape(P, -1)
    )

    in_maps = []
    owners = []
    for c in range(N_CORES):
        xt, xh, yoff, owner = _route_core(
            np.ascontiguousarray(x[c * TOK : (c + 1) * TOK]),
            gi[c * TOK : (c + 1) * TOK],
            cap,
        )
        in_maps.append(
            {"xt": xt, "xh": xh, "wh": wh, "w": weight, "b": bias, "yo": yoff}
        )
        owners.append(owner)
    res = run_bass_kernel_spmd(nc, in_maps, core_ids=list(range(N_CORES)))
    LAST_RESULTS = res

    out = np.empty((BATCH, DOUT), dtype=np.float32)
    for c in range(N_CORES):
        out_c = out[c * TOK : (c + 1) * TOK]
        for k in range(NOUT):
            m = owners[c] == k
            out_c[m] = res.results[c][f"out{k}"][m].astype(np.float32)
    return out
